# revision 19
# baseline (speedup 1.0000x reference)
"""AttentionPooling TRN2 kernel: 8-core data-parallel over flattened (B*N) points.

Math (per point n with k=16 neighbors, C=512 channels):
  logits = x @ w_score.T            (per-channel attention logits)
  scores = softmax_k(logits)        (softmax over the k axis, per channel)
  pooled = sum_k x * scores
  y      = relu((pooled @ w_conv.T - mean) * gamma/sqrt(var+eps) + beta)

Device mapping (per core, 2048 n-points = 32768 (n,k) rows):
  - x rows (pt=(n,k) on partitions, c on free) feed the elementwise product.
  - mm1 uses bf16 copies of x transposed ON THE HOST (c on partitions) as the
    stationary operand: logits = xT.T @ w_score.T.  Device-side xbar
    transposes were the bottleneck (~64GB/s effective) and also corrupt
    under concurrent f32r/ACT-queue DMA traffic, so they are avoided.
  - softmax-over-k reductions run on the TensorEngine as matmuls with a
    0/1 group matrix G (k groups live in partition dim), accumulating 16
    chunks into one packed (128 n, 512 c) PSUM tile.
  - BN is folded into w_conv (scale) + a rank-1 bias matmul; ReLU on DVE.
  - fp16 is used for everything except the bf16 mm1 and fp32 accumulators:
    exact for the 0/1 G matrix, ~2^-11 rounding elsewhere, full-rate matmuls
    with overlappable weight loads, and 2x DVE modes.
"""
import numpy as np
import ml_dtypes

B, N, K, C, COUT = 4, 4096, 16, 512, 512
NCORES = 8
PTS_PER_CORE = B * N * K // NCORES      # 32768
NROWS_PER_CORE = B * N // NCORES        # 2048 n-points
NSB = NROWS_PER_CORE // 128             # 16 super-blocks of 128 n
NCHUNK = 16                             # chunks of 128 (n,k) rows per super-block
BN_EPS = 1e-5

_cached = {}


def _build():
    import concourse.bacc as bacc
    import concourse.mybir as mybir
    import concourse.tile as tile

    F32, F16, BF16 = mybir.dt.float32, mybir.dt.float16, mybir.dt.bfloat16
    ACT = mybir.ActivationFunctionType

    nc = bacc.Bacc("TRN2", target_bir_lowering=False, debug=False, num_devices=NCORES)
    xf = nc.dram_tensor("xf", [PTS_PER_CORE, C], F32, kind="ExternalInput")
    xbT = nc.dram_tensor("xbT", [4 * 128, PTS_PER_CORE], BF16, kind="ExternalInput")
    wst = nc.dram_tensor("wst", [C, C], BF16, kind="ExternalInput")
    wc2t = nc.dram_tensor("wc2t", [C, COUT], F32, kind="ExternalInput")
    bias2 = nc.dram_tensor("bias2", [1, COUT], F32, kind="ExternalInput")
    ones = nc.dram_tensor("ones", [1, 128], F32, kind="ExternalInput")
    gmat = nc.dram_tensor("gmat", [128, 128 * NCHUNK], F32, kind="ExternalInput")
    ident = nc.dram_tensor("ident", [128, 128], F32, kind="ExternalInput")
    y = nc.dram_tensor("y", [NROWS_PER_CORE, COUT], F32, kind="ExternalOutput")

    with tile.TileContext(nc) as tc:
        with (
            tc.tile_pool(name="const", bufs=1) as cp,
            tc.tile_pool(name="xT", bufs=3) as xtp,
            tc.tile_pool(name="xsb", bufs=2) as xsp,
            tc.tile_pool(name="work", bufs=13) as wp,
            tc.tile_pool(name="tail", bufs=2) as tp,
            tc.tile_pool(name="pl", bufs=3, space="PSUM") as pslp,
            tc.tile_pool(name="psacc", bufs=2, space="PSUM") as psa,
            tc.tile_pool(name="pstail", bufs=1, space="PSUM") as pst,
        ):
            # ---- constants (f16 ones produced via ACT copy from f32 staging) ----
            wst_t = [cp.tile([128, C], BF16, tag=f"wst{i}", name=f"wst{i}") for i in range(4)]
            for i in range(4):
                nc.sync.dma_start(wst_t[i][:], wst[128 * i:128 * (i + 1), :])
            wc2t_t = [cp.tile([128, COUT], F16, tag=f"wc2t{i}", name=f"wc2t{i}") for i in range(4)]
            bias2_t = cp.tile([1, COUT], F16, tag="bias2")
            ones_t = cp.tile([1, 128], F16, tag="ones")
            g_t = cp.tile([128, 128 * NCHUNK], F16, tag="g")
            id_t = cp.tile([128, 128], F16, tag="ident")
            with tc.tile_pool(name="staging", bufs=1) as stp:
                wc2t_f = [stp.tile([128, COUT], F32, tag=f"wc2tf{i}", name=f"wc2tf{i}") for i in range(4)]
                for i in range(4):
                    nc.gpsimd.dma_start(wc2t_f[i][:], wc2t[128 * i:128 * (i + 1), :])
                    nc.scalar.copy(wc2t_t[i][:], wc2t_f[i][:])
                bias2_f = stp.tile([1, COUT], F32, tag="bias2f")
                nc.gpsimd.dma_start(bias2_f[:], bias2[:])
                nc.scalar.copy(bias2_t[:], bias2_f[:])
                ones_f = stp.tile([1, 128], F32, tag="onesf")
                nc.gpsimd.dma_start(ones_f[:], ones[:])
                nc.scalar.copy(ones_t[:], ones_f[:])
                g_f = stp.tile([128, 128 * NCHUNK], F32, tag="gf")
                nc.gpsimd.dma_start(g_f[:], gmat[:])
                nc.scalar.copy(g_t[:], g_f[:])
                id_f = stp.tile([128, 128], F32, tag="identf")
                nc.gpsimd.dma_start(id_f[:], ident[:])
                nc.scalar.copy(id_t[:], id_f[:])

            # Software-pipelined chunk loop over all NSB*NCHUNK chunks.
            # - G (reduction) matmuls for chunk c are emitted GDELAY chunks
            #   late so the PE never waits on that chunk's exp/mul chain.
            # - x (f32) is loaded in half-super-blocks (2 MB) with separate
            #   tile tags so slot recycling never delays the next prefetch.
            # - each super-block tail is emitted as small scheduled steps so
            #   the reciprocal never blocks the DVE FIFO for long and the PE
            #   tail matmuls land well after their inputs are ready.
            GDELAY = 10
            TOTAL = NSB * NCHUNK
            xT_all = {}
            xsb_all = {}
            chunk_et = {}
            acc = {}
            sched_steps = {}

            def issue_transposes(sb):
                xT = [
                    xtp.tile([128, 2048], BF16, tag=f"xT{i}", name=f"xT{i}_{sb}")
                    for i in range(4)
                ]
                row0 = 2048 * sb
                for i in range(4):
                    nc.sync.dma_start(
                        xT[i][:], xbT[128 * i:128 * (i + 1), row0:row0 + 2048]
                    )
                xT_all[sb] = xT

            def issue_xh(sb, h):
                row0 = 2048 * sb + 1024 * h
                xh = xsp.tile([128, 8, C], F32, tag=f"xsb{h}", name=f"xh{sb}_{h}")
                src_ap = xf[row0:row0 + 1024, :].rearrange("(ch p) c -> p ch c", p=128)
                nc.gpsimd.dma_start(xh[:], src_ap)
                xsb_all[(sb, h)] = xh

            def make_tail_steps(sb):
                pe_sum, pt_sum = acc.pop(sb)
                inv_t = tp.tile([128, C], F32, tag="inv", name=f"inv{sb}")
                pooled = tp.tile([128, C], F16, tag="pooled", name=f"pooled{sb}")
                state = {}

                def norm_step(i):
                    def f():
                        sl = slice(64 * i, 64 * (i + 1))
                        nc.vector.reciprocal(inv_t[:, sl], pe_sum[:, sl])
                        nc.vector.tensor_mul(pooled[:, sl], pt_sum[:, sl], inv_t[:, sl])
                    return f

                def transpose_step():
                    ppT = pst.tile([128, C], F16, tag="pstail", name=f"ppT{sb}")
                    for i in range(4):
                        nc.tensor.transpose(
                            ppT[:, 128 * i:128 * (i + 1)],
                            pooled[:, 128 * i:128 * (i + 1)],
                            id_t[:],
                        )
                    pT = tp.tile([128, C], F16, tag="pT", name=f"pT{sb}")
                    nc.vector.tensor_copy(pT[:], ppT[:])
                    state["pT"] = pT

                def mm2_step():
                    pT = state["pT"]
                    py = pst.tile([128, COUT], F32, tag="pstail", name=f"py{sb}")
                    for i in range(4):
                        nc.tensor.matmul(
                            py[:],
                            pT[:, 128 * i:128 * (i + 1)],
                            wc2t_t[i][:],
                            start=(i == 0),
                            stop=False,
                        )
                    nc.tensor.matmul(
                        py[:], ones_t[:], bias2_t[:], start=False, stop=True
                    )
                    y_t = tp.tile([128, COUT], F32, tag="yt", name=f"yt{sb}")
                    nc.vector.tensor_scalar_max(y_t[:], py[:], 0.0)
                    nc.gpsimd.dma_start(y[128 * sb:128 * (sb + 1), :], y_t[:])

                return [norm_step(i) for i in range(8)] + [transpose_step, mm2_step]



            issue_transposes(0)
            issue_transposes(1)
            issue_transposes(2)
            issue_xh(0, 0)
            issue_xh(0, 1)
            for c in range(TOTAL + GDELAY + 16):
                sb, j = divmod(c, NCHUNK)
                if c < TOTAL:
                    if j == 0 and sb + 1 < NSB:
                        issue_xh(sb + 1, 0)
                    if j == 8 and sb + 1 < NSB:
                        issue_xh(sb + 1, 1)
                    if j == 1 and sb + 3 < NSB:
                        issue_transposes(sb + 3)
                    xT = xT_all[sb]
                    pl = pslp.tile([128, C], F32, tag="pl", name=f"pl{c}")
                    for i in range(4):
                        nc.tensor.matmul(
                            pl[:],
                            xT[i][:, 128 * j:128 * (j + 1)],
                            wst_t[i][:],
                            start=(i == 0),
                            stop=(i == 3),
                        )
                    ej = wp.tile([128, C], F16, tag="ej", name=f"ej{c}")
                    nc.scalar.activation(ej[:], pl[:], ACT.Exp)
                    tj = wp.tile([128, C], F16, tag="tj", name=f"tj{c}")
                    nc.vector.tensor_mul(tj[:], xsb_all[(sb, j // 8)][:, j % 8, :], ej[:])
                    chunk_et[c] = (ej, tj)
                d = c - GDELAY
                if 0 <= d < TOTAL:
                    dsb, dj = divmod(d, NCHUNK)
                    if dj == 0:
                        acc[dsb] = (
                            psa.tile([128, C], F32, tag="esum", name=f"esum{dsb}"),
                            psa.tile([128, C], F32, tag="tsum", name=f"tsum{dsb}"),
                        )
                    pe_sum, pt_sum = acc[dsb]
                    ej, tj = chunk_et.pop(d)
                    gj = g_t[:, 128 * dj:128 * (dj + 1)]
                    nc.tensor.matmul(
                        pe_sum[:], gj, ej[:], start=(dj == 0), stop=(dj == NCHUNK - 1)
                    )
                    nc.tensor.matmul(
                        pt_sum[:], gj, tj[:], start=(dj == 0), stop=(dj == NCHUNK - 1)
                    )
                    if dj == NCHUNK - 1:
                        steps = make_tail_steps(dsb)
                        for off, st in zip((1, 2, 3, 4, 5, 6, 7, 8, 10, 13), steps):
                            sched_steps.setdefault(c + off, []).append(st)
                for st in sched_steps.pop(c, []):
                    st()
    nc.compile()
    return nc


def _get_nc():
    if "nc" not in _cached:
        _cached["nc"] = _build()
    return _cached["nc"]


def _host_prep(x, w_score, w_conv, bn_gamma, bn_beta, bn_mean, bn_var):
    x = np.ascontiguousarray(np.asarray(x, dtype=np.float32)).reshape(B * N * K, C)
    w_score = np.asarray(w_score, dtype=np.float32)
    w_conv = np.asarray(w_conv, dtype=np.float32)
    inv = np.asarray(bn_gamma, dtype=np.float64) / np.sqrt(
        np.asarray(bn_var, dtype=np.float64) + BN_EPS
    )
    wc2 = w_conv.astype(np.float64) * inv[:, None]
    bias2 = (
        np.asarray(bn_beta, dtype=np.float64)
        - np.asarray(bn_mean, dtype=np.float64) * inv
    )
    g = np.zeros((128, 128 * NCHUNK), dtype=np.float32)
    for j in range(NCHUNK):
        for p in range(128):
            g[p, 128 * j + 8 * j + p // 16] = 1.0
    common = {
        "wst": np.ascontiguousarray(w_score.T).astype(ml_dtypes.bfloat16),
        "wc2t": np.ascontiguousarray(wc2.T).astype(np.float32),
        "bias2": bias2.reshape(1, COUT).astype(np.float32),
        "ones": np.ones((1, 128), dtype=np.float32),
        "gmat": g,
        "ident": np.eye(128, dtype=np.float32),
    }
    xb = x.astype(ml_dtypes.bfloat16)
    in_maps = []
    for c in range(NCORES):
        sl = slice(PTS_PER_CORE * c, PTS_PER_CORE * (c + 1))
        # host-transposed bf16: strip i holds channels 128i..128i+128 on the
        # partition axis so mm1's stationary loads need no on-device transpose.
        xbs = np.ascontiguousarray(xb[sl].T).reshape(4 * 128, PTS_PER_CORE)
        in_maps.append({"xf": x[sl], "xbT": xbs, **common})
    return in_maps


def kernel(x, w_score, w_conv, bn_gamma, bn_beta, bn_mean, bn_var):
    from concourse.bass_utils import run_bass_kernel_spmd

    nc = _get_nc()
    in_maps = _host_prep(x, w_score, w_conv, bn_gamma, bn_beta, bn_mean, bn_var)
    res = run_bass_kernel_spmd(nc, in_maps, core_ids=list(range(NCORES)))
    out = np.concatenate([res.results[c]["y"] for c in range(NCORES)], axis=0)
    return out.reshape(B, N, COUT).astype(np.float32)


# revision 22
# speedup vs baseline: 1.1636x; 1.1636x over previous
"""AttentionPooling TRN2 kernel: 8-core data-parallel over flattened (B*N) points.

Math (per point n with k=16 neighbors, C=512 channels):
  logits = x @ w_score.T            (per-channel attention logits)
  scores = softmax_k(logits)        (softmax over the k axis, per channel)
  pooled = sum_k x * scores
  y      = relu((pooled @ w_conv.T - mean) * gamma/sqrt(var+eps) + beta)

Device mapping (per core, 2048 n-points = 32768 (n,k) rows):
  - x rows (pt=(n,k) on partitions, c on free) feed the elementwise product.
  - mm1 uses bf16 copies of x transposed ON THE HOST (c on partitions) as the
    stationary operand: logits = xT.T @ w_score.T.  Device-side xbar
    transposes were the bottleneck (~64GB/s effective) and also corrupt
    under concurrent f32r/ACT-queue DMA traffic, so they are avoided.
  - softmax-over-k reductions run on the TensorEngine as matmuls with a
    0/1 group matrix G (k groups live in partition dim), accumulating 16
    chunks into one packed (128 n, 512 c) PSUM tile.
  - BN is folded into w_conv (scale) + a rank-1 bias matmul; ReLU on DVE.
  - fp16 is used for everything except the bf16 mm1 and fp32 accumulators:
    exact for the 0/1 G matrix, ~2^-11 rounding elsewhere, full-rate matmuls
    with overlappable weight loads, and 2x DVE modes.
"""
import numpy as np
import ml_dtypes

B, N, K, C, COUT = 4, 4096, 16, 512, 512
NCORES = 8
PTS_PER_CORE = B * N * K // NCORES      # 32768
NROWS_PER_CORE = B * N // NCORES        # 2048 n-points
NSB = NROWS_PER_CORE // 128             # 16 super-blocks of 128 n
NCHUNK = 16                             # chunks of 128 (n,k) rows per super-block
BN_EPS = 1e-5

_cached = {}


def _build():
    import concourse.bacc as bacc
    import concourse.mybir as mybir
    import concourse.tile as tile

    F32, F16, BF16 = mybir.dt.float32, mybir.dt.float16, mybir.dt.bfloat16
    ACT = mybir.ActivationFunctionType

    nc = bacc.Bacc("TRN2", target_bir_lowering=False, debug=False, num_devices=NCORES)
    xf = nc.dram_tensor("xf", [PTS_PER_CORE, C], F32, kind="ExternalInput")
    xbT = nc.dram_tensor("xbT", [4 * 128, PTS_PER_CORE], BF16, kind="ExternalInput")
    wst = nc.dram_tensor("wst", [C, C], BF16, kind="ExternalInput")
    wc2t = nc.dram_tensor("wc2t", [C, COUT], F32, kind="ExternalInput")
    bias2 = nc.dram_tensor("bias2", [1, COUT], F32, kind="ExternalInput")
    ones = nc.dram_tensor("ones", [1, 128], F32, kind="ExternalInput")
    gmat = nc.dram_tensor("gmat", [128, 128 * NCHUNK], F32, kind="ExternalInput")
    ident = nc.dram_tensor("ident", [128, 128], F32, kind="ExternalInput")
    y = nc.dram_tensor("y", [NROWS_PER_CORE, COUT], F32, kind="ExternalOutput")

    with tile.TileContext(nc) as tc:
        with (
            tc.tile_pool(name="const", bufs=1) as cp,
            tc.tile_pool(name="xT", bufs=3) as xtp,
            tc.tile_pool(name="xsb", bufs=2) as xsp,
            tc.tile_pool(name="work", bufs=13) as wp,
            tc.tile_pool(name="tail", bufs=2) as tp,
            tc.tile_pool(name="pl", bufs=3, space="PSUM") as pslp,
            tc.tile_pool(name="psacc", bufs=2, space="PSUM") as psa,
            tc.tile_pool(name="pstail", bufs=1, space="PSUM") as pst,
        ):
            # ---- constants (f16 ones produced via ACT copy from f32 staging) ----
            wst_t = [cp.tile([128, C], BF16, tag=f"wst{i}", name=f"wst{i}") for i in range(4)]
            for i in range(4):
                nc.sync.dma_start(wst_t[i][:], wst[128 * i:128 * (i + 1), :])
            wc2t_t = [cp.tile([128, COUT], F16, tag=f"wc2t{i}", name=f"wc2t{i}") for i in range(4)]
            bias2_t = cp.tile([1, COUT], F16, tag="bias2")
            ones_t = cp.tile([1, 128], F16, tag="ones")
            g_t = cp.tile([128, 128 * NCHUNK], F16, tag="g")
            id_t = cp.tile([128, 128], F16, tag="ident")
            with tc.tile_pool(name="staging", bufs=1) as stp:
                wc2t_f = [stp.tile([128, COUT], F32, tag=f"wc2tf{i}", name=f"wc2tf{i}") for i in range(4)]
                for i in range(4):
                    nc.gpsimd.dma_start(wc2t_f[i][:], wc2t[128 * i:128 * (i + 1), :])
                    nc.scalar.copy(wc2t_t[i][:], wc2t_f[i][:])
                bias2_f = stp.tile([1, COUT], F32, tag="bias2f")
                nc.gpsimd.dma_start(bias2_f[:], bias2[:])
                nc.scalar.copy(bias2_t[:], bias2_f[:])
                ones_f = stp.tile([1, 128], F32, tag="onesf")
                nc.gpsimd.dma_start(ones_f[:], ones[:])
                nc.scalar.copy(ones_t[:], ones_f[:])
                g_f = stp.tile([128, 128 * NCHUNK], F32, tag="gf")
                nc.gpsimd.dma_start(g_f[:], gmat[:])
                nc.scalar.copy(g_t[:], g_f[:])
                id_f = stp.tile([128, 128], F32, tag="identf")
                nc.gpsimd.dma_start(id_f[:], ident[:])
                nc.scalar.copy(id_t[:], id_f[:])

            # Software-pipelined chunk loop over all NSB*NCHUNK chunks.
            # - G (reduction) matmuls for chunk c are emitted GDELAY chunks
            #   late so the PE never waits on that chunk's exp/mul chain.
            # - x (f32) is loaded in half-super-blocks (2 MB) with separate
            #   tile tags so slot recycling never delays the next prefetch.
            # - each super-block tail is emitted as small scheduled steps so
            #   the reciprocal never blocks the DVE FIFO for long and the PE
            #   tail matmuls land well after their inputs are ready.
            GDELAY = 10
            TOTAL = NSB * NCHUNK
            xT_all = {}
            xsb_all = {}
            chunk_et = {}
            acc = {}
            sched_steps = {}

            def issue_transposes(sb):
                xT = [
                    xtp.tile([128, 2048], BF16, tag=f"xT{i}", name=f"xT{i}_{sb}")
                    for i in range(4)
                ]
                row0 = 2048 * sb
                for i in range(4):
                    nc.sync.dma_start(
                        xT[i][:], xbT[128 * i:128 * (i + 1), row0:row0 + 2048]
                    )
                xT_all[sb] = xT

            def issue_xh(sb, h):
                row0 = 2048 * sb + 1024 * h
                xh = xsp.tile([128, 8, C], F32, tag=f"xsb{h}", name=f"xh{sb}_{h}")
                src_ap = xf[row0:row0 + 1024, :].rearrange("(ch p) c -> p ch c", p=128)
                nc.gpsimd.dma_start(xh[:], src_ap)
                xsb_all[(sb, h)] = xh

            def make_tail_steps(sb):
                pe_sum, pt_sum = acc.pop(sb)
                inv_t = tp.tile([128, C], F32, tag="inv", name=f"inv{sb}")
                pooled = tp.tile([128, C], F16, tag="pooled", name=f"pooled{sb}")
                state = {}

                def norm_step(i):
                    def f():
                        sl = slice(64 * i, 64 * (i + 1))
                        nc.vector.reciprocal(inv_t[:, sl], pe_sum[:, sl])
                        nc.vector.tensor_mul(pooled[:, sl], pt_sum[:, sl], inv_t[:, sl])
                    return f

                def transpose_step():
                    ppT = pst.tile([128, C], F16, tag="pstail", name=f"ppT{sb}")
                    for i in range(4):
                        nc.tensor.transpose(
                            ppT[:, 128 * i:128 * (i + 1)],
                            pooled[:, 128 * i:128 * (i + 1)],
                            id_t[:],
                        )
                    pT = tp.tile([128, C], F16, tag="pT", name=f"pT{sb}")
                    nc.vector.tensor_copy(pT[:], ppT[:])
                    state["pT"] = pT

                def mm2_step():
                    pT = state["pT"]
                    py = pst.tile([128, COUT], F32, tag="pstail", name=f"py{sb}")
                    for i in range(4):
                        nc.tensor.matmul(
                            py[:],
                            pT[:, 128 * i:128 * (i + 1)],
                            wc2t_t[i][:],
                            start=(i == 0),
                            stop=False,
                        )
                    nc.tensor.matmul(
                        py[:], ones_t[:], bias2_t[:], start=False, stop=True
                    )
                    y_t = tp.tile([128, COUT], F32, tag="yt", name=f"yt{sb}")
                    nc.vector.tensor_scalar_max(y_t[:], py[:], 0.0)
                    nc.gpsimd.dma_start(y[128 * sb:128 * (sb + 1), :], y_t[:])

                return [norm_step(i) for i in range(8)] + [transpose_step, mm2_step]



            issue_transposes(0)
            issue_transposes(1)
            issue_transposes(2)
            issue_xh(0, 0)
            issue_xh(0, 1)
            for c in range(TOTAL + GDELAY + 16):
                sb, j = divmod(c, NCHUNK)
                if c < TOTAL:
                    if j == 0 and sb + 1 < NSB:
                        issue_xh(sb + 1, 0)
                    if j == 8 and sb + 1 < NSB:
                        issue_xh(sb + 1, 1)
                    if j == 1 and sb + 3 < NSB:
                        issue_transposes(sb + 3)
                    xT = xT_all[sb]
                    pl = pslp.tile([128, C], F32, tag="pl", name=f"pl{c}")
                    for i in range(4):
                        nc.tensor.matmul(
                            pl[:],
                            xT[i][:, 128 * j:128 * (j + 1)],
                            wst_t[i][:],
                            start=(i == 0),
                            stop=(i == 3),
                        )
                    ej = wp.tile([128, C], F16, tag="ej", name=f"ej{c}")
                    nc.scalar.activation(ej[:], pl[:], ACT.Exp)
                    tj = wp.tile([128, C], F16, tag="tj", name=f"tj{c}")
                    nc.vector.tensor_mul(tj[:], xsb_all[(sb, j // 8)][:, j % 8, :], ej[:])
                    chunk_et[c] = (ej, tj)
                d = c - GDELAY
                if 0 <= d < TOTAL:
                    dsb, dj = divmod(d, NCHUNK)
                    if dj == 0:
                        acc[dsb] = (
                            psa.tile([128, C], F32, tag="esum", name=f"esum{dsb}"),
                            psa.tile([128, C], F32, tag="tsum", name=f"tsum{dsb}"),
                        )
                    pe_sum, pt_sum = acc[dsb]
                    ej, tj = chunk_et.pop(d)
                    gj = g_t[:, 128 * dj:128 * (dj + 1)]
                    nc.tensor.matmul(
                        pe_sum[:], gj, ej[:], start=(dj == 0), stop=(dj == NCHUNK - 1)
                    )
                    nc.tensor.matmul(
                        pt_sum[:], gj, tj[:], start=(dj == 0), stop=(dj == NCHUNK - 1)
                    )
                    if dj == NCHUNK - 1:
                        steps = make_tail_steps(dsb)
                        for off, st in zip((1, 2, 3, 4, 5, 6, 7, 8, 10, 13), steps):
                            sched_steps.setdefault(c + off, []).append(st)
                for st in sched_steps.pop(c, []):
                    st()
    nc.compile()
    return nc


def _get_nc():
    if "nc" not in _cached:
        _cached["nc"] = _build()
    return _cached["nc"]


def _host_prep(x, w_score, w_conv, bn_gamma, bn_beta, bn_mean, bn_var):
    x = np.ascontiguousarray(np.asarray(x, dtype=np.float32)).reshape(B * N * K, C)
    w_score = np.asarray(w_score, dtype=np.float32)
    w_conv = np.asarray(w_conv, dtype=np.float32)
    inv = np.asarray(bn_gamma, dtype=np.float64) / np.sqrt(
        np.asarray(bn_var, dtype=np.float64) + BN_EPS
    )
    wc2 = w_conv.astype(np.float64) * inv[:, None]
    bias2 = (
        np.asarray(bn_beta, dtype=np.float64)
        - np.asarray(bn_mean, dtype=np.float64) * inv
    )
    g = np.zeros((128, 128 * NCHUNK), dtype=np.float32)
    for j in range(NCHUNK):
        for p in range(128):
            g[p, 128 * j + 8 * j + p // 16] = 1.0
    common = {
        "wst": np.ascontiguousarray(w_score.T).astype(ml_dtypes.bfloat16),
        "wc2t": np.ascontiguousarray(wc2.T).astype(np.float32),
        "bias2": bias2.reshape(1, COUT).astype(np.float32),
        "ones": np.ones((1, 128), dtype=np.float32),
        "gmat": g,
        "ident": np.eye(128, dtype=np.float32),
    }
    xb = x.astype(ml_dtypes.bfloat16)
    in_maps = []
    for c in range(NCORES):
        sl = slice(PTS_PER_CORE * c, PTS_PER_CORE * (c + 1))
        # host-transposed bf16: strip i holds channels 128i..128i+128 on the
        # partition axis so mm1's stationary loads need no on-device transpose.
        xbs = np.ascontiguousarray(xb[sl].T).reshape(4 * 128, PTS_PER_CORE)
        in_maps.append({"xf": x[sl], "xbT": xbs, **common})
    return in_maps


def kernel(x, w_score, w_conv, bn_gamma, bn_beta, bn_mean, bn_var):
    from concourse.bass_utils import run_bass_kernel_spmd

    nc = _get_nc()
    in_maps = _host_prep(x, w_score, w_conv, bn_gamma, bn_beta, bn_mean, bn_var)
    res = run_bass_kernel_spmd(nc, in_maps, core_ids=list(range(NCORES)))
    out = np.concatenate([res.results[c]["y"] for c in range(NCORES)], axis=0)
    return out.reshape(B, N, COUT).astype(np.float32)


# revision 24
# speedup vs baseline: 1.2889x; 1.1077x over previous
"""AttentionPooling TRN2 kernel: 8-core data-parallel over flattened (B*N) points.

Math (per point n with k=16 neighbors, C=512 channels):
  logits = x @ w_score.T            (per-channel attention logits)
  scores = softmax_k(logits)        (softmax over the k axis, per channel)
  pooled = sum_k x * scores
  y      = relu((pooled @ w_conv.T - mean) * gamma/sqrt(var+eps) + beta)

Device mapping (per core, 2048 n-points = 32768 (n,k) rows):
  - x rows (pt=(n,k) on partitions, c on free) feed the elementwise product.
  - mm1 uses bf16 copies of x transposed ON THE HOST (c on partitions) as the
    stationary operand: logits = xT.T @ w_score.T.  Device-side xbar
    transposes were the bottleneck (~64GB/s effective) and also corrupt
    under concurrent f32r/ACT-queue DMA traffic, so they are avoided.
  - softmax-over-k reductions run on the TensorEngine as matmuls with a
    0/1 group matrix G (k groups live in partition dim), accumulating 16
    chunks into one packed (128 n, 512 c) PSUM tile.
  - BN is folded into w_conv (scale) + a rank-1 bias matmul; ReLU on DVE.
  - fp16 is used for everything except the bf16 mm1 and fp32 accumulators:
    exact for the 0/1 G matrix, ~2^-11 rounding elsewhere, full-rate matmuls
    with overlappable weight loads, and 2x DVE modes.
"""
import numpy as np
import ml_dtypes

B, N, K, C, COUT = 4, 4096, 16, 512, 512
NCORES = 8
PTS_PER_CORE = B * N * K // NCORES      # 32768
NROWS_PER_CORE = B * N // NCORES        # 2048 n-points
NSB = NROWS_PER_CORE // 128             # 16 super-blocks of 128 n
NCHUNK = 16                             # chunks of 128 (n,k) rows per super-block
BN_EPS = 1e-5

_cached = {}


def _build():
    import concourse.bacc as bacc
    import concourse.mybir as mybir
    import concourse.tile as tile

    F32, F16, BF16 = mybir.dt.float32, mybir.dt.float16, mybir.dt.bfloat16
    ACT = mybir.ActivationFunctionType

    nc = bacc.Bacc("TRN2", target_bir_lowering=False, debug=False, num_devices=NCORES)
    xf = nc.dram_tensor("xf", [PTS_PER_CORE, C], F32, kind="ExternalInput")
    xbT = nc.dram_tensor("xbT", [4 * 128, PTS_PER_CORE], BF16, kind="ExternalInput")
    wst = nc.dram_tensor("wst", [C, C], BF16, kind="ExternalInput")
    wc2t = nc.dram_tensor("wc2t", [C, COUT], F32, kind="ExternalInput")
    bias2 = nc.dram_tensor("bias2", [1, COUT], F32, kind="ExternalInput")
    ones = nc.dram_tensor("ones", [1, 128], F32, kind="ExternalInput")
    gmat = nc.dram_tensor("gmat", [128, 128], F32, kind="ExternalInput")
    ident = nc.dram_tensor("ident", [128, 128], F32, kind="ExternalInput")
    y = nc.dram_tensor("y", [NROWS_PER_CORE, COUT], F32, kind="ExternalOutput")

    with tile.TileContext(nc) as tc:
        with (
            tc.tile_pool(name="const", bufs=1) as cp,
            tc.tile_pool(name="xT", bufs=3) as xtp,
            tc.tile_pool(name="xsb", bufs=2) as xsp,
            tc.tile_pool(name="work", bufs=18) as wp,
            tc.tile_pool(name="tail", bufs=2) as tp,
            tc.tile_pool(name="pl", bufs=3, space="PSUM") as pslp,
            tc.tile_pool(name="psacc", bufs=2, space="PSUM") as psa,
            tc.tile_pool(name="pstail", bufs=1, space="PSUM") as pst,
        ):
            # ---- constants (f16 ones produced via ACT copy from f32 staging) ----
            wst_t = [cp.tile([128, C], BF16, tag=f"wst{i}", name=f"wst{i}") for i in range(4)]
            for i in range(4):
                nc.sync.dma_start(wst_t[i][:], wst[128 * i:128 * (i + 1), :])
            wc2t_t = [cp.tile([128, COUT], F16, tag=f"wc2t{i}", name=f"wc2t{i}") for i in range(4)]
            bias2_t = cp.tile([1, COUT], F16, tag="bias2")
            ones_t = cp.tile([1, 128], F16, tag="ones")
            g_t = cp.tile([128, 128], F16, tag="g")
            id_t = cp.tile([128, 128], F16, tag="ident")
            zcol = cp.tile([1, 128], F16, tag="zcol")
            nc.vector.memset(zcol[:], 0.0)
            zrow = cp.tile([1, COUT], F16, tag="zrow")
            nc.vector.memset(zrow[:], 0.0)
            with tc.tile_pool(name="staging", bufs=1) as stp:
                wc2t_f = [stp.tile([128, COUT], F32, tag=f"wc2tf{i}", name=f"wc2tf{i}") for i in range(4)]
                for i in range(4):
                    nc.gpsimd.dma_start(wc2t_f[i][:], wc2t[128 * i:128 * (i + 1), :])
                    nc.scalar.copy(wc2t_t[i][:], wc2t_f[i][:])
                bias2_f = stp.tile([1, COUT], F32, tag="bias2f")
                nc.gpsimd.dma_start(bias2_f[:], bias2[:])
                nc.scalar.copy(bias2_t[:], bias2_f[:])
                ones_f = stp.tile([1, 128], F32, tag="onesf")
                nc.gpsimd.dma_start(ones_f[:], ones[:])
                nc.scalar.copy(ones_t[:], ones_f[:])
                g_f = stp.tile([128, 128], F32, tag="gf")
                nc.gpsimd.dma_start(g_f[:], gmat[:])
                nc.scalar.copy(g_t[:], g_f[:])
                id_f = stp.tile([128, 128], F32, tag="identf")
                nc.gpsimd.dma_start(id_f[:], ident[:])
                nc.scalar.copy(id_t[:], id_f[:])

            # Software-pipelined chunk loop over all NSB*NCHUNK chunks.
            # - G (reduction) matmuls for chunk c are emitted GDELAY chunks
            #   late so the PE never waits on that chunk's exp/mul chain.
            # - x (f32) is loaded in half-super-blocks (2 MB) with separate
            #   tile tags so slot recycling never delays the next prefetch.
            # - each super-block tail is emitted as small scheduled steps so
            #   the reciprocal never blocks the DVE FIFO for long and the PE
            #   tail matmuls land well after their inputs are ready.
            GDELAY = 4
            TOTAL = NSB * NCHUNK
            xT_all = {}
            xsb_all = {}
            chunk_et = {}
            acc = {}
            sched_steps = {}

            def issue_transposes(sb):
                xT = [
                    xtp.tile([128, 2048], BF16, tag=f"xT{i}", name=f"xT{i}_{sb}")
                    for i in range(4)
                ]
                row0 = 2048 * sb
                for i in range(4):
                    nc.sync.dma_start(
                        xT[i][:], xbT[128 * i:128 * (i + 1), row0:row0 + 2048]
                    )
                xT_all[sb] = xT

            def issue_xh(sb, h):
                row0 = 2048 * sb + 1024 * h
                xh = xsp.tile([128, 8, C], F32, tag=f"xsb{h}", name=f"xh{sb}_{h}")
                src_ap = xf[row0:row0 + 1024, :].rearrange("(ch p) c -> p ch c", p=128)
                nc.gpsimd.dma_start(xh[:], src_ap)
                xsb_all[(sb, h)] = xh

            def make_tail_steps(sb):
                pe_sum, pt_sum = acc.pop(sb)
                inv_t = tp.tile([128, C], F32, tag="inv", name=f"inv{sb}")
                pooled = tp.tile([128, C], F16, tag="pooled", name=f"pooled{sb}")
                state = {}

                def norm_step(i):
                    def f():
                        sl = slice(64 * i, 64 * (i + 1))
                        nc.vector.reciprocal(inv_t[:, sl], pe_sum[:, sl])
                        nc.vector.tensor_mul(pooled[:, sl], pt_sum[:, sl], inv_t[:, sl])
                    return f

                def transpose_step():
                    ppT = pst.tile([128, C], F16, tag="pstail", name=f"ppT{sb}")
                    for i in range(4):
                        nc.tensor.transpose(
                            ppT[:, 128 * i:128 * (i + 1)],
                            pooled[:, 128 * i:128 * (i + 1)],
                            id_t[:],
                        )
                    pT = tp.tile([128, C], F16, tag="pT", name=f"pT{sb}")
                    nc.vector.tensor_copy(pT[:], ppT[:])
                    state["pT"] = pT

                def mm2_step():
                    pT = state["pT"]
                    py = pst.tile([128, COUT], F32, tag="pstail", name=f"py{sb}")
                    for i in range(4):
                        nc.tensor.matmul(
                            py[:],
                            pT[:, 128 * i:128 * (i + 1)],
                            wc2t_t[i][:],
                            start=(i == 0),
                            stop=False,
                        )
                    nc.tensor.matmul(
                        py[:], ones_t[:], bias2_t[:], start=False, stop=True
                    )
                    y_t = tp.tile([128, COUT], F32, tag="yt", name=f"yt{sb}")
                    nc.vector.tensor_scalar_max(y_t[:], py[:], 0.0)
                    nc.gpsimd.dma_start(y[128 * sb:128 * (sb + 1), :], y_t[:])

                return [norm_step(i) for i in range(8)] + [transpose_step, mm2_step]



            issue_transposes(0)
            issue_transposes(1)
            issue_transposes(2)
            issue_xh(0, 0)
            issue_xh(0, 1)
            for c in range(TOTAL + GDELAY + 16):
                sb, j = divmod(c, NCHUNK)
                if c < TOTAL:
                    if j == 0 and sb + 1 < NSB:
                        issue_xh(sb + 1, 0)
                    if j == 8 and sb + 1 < NSB:
                        issue_xh(sb + 1, 1)
                    if j == 1 and sb + 3 < NSB:
                        issue_transposes(sb + 3)
                    xT = xT_all[sb]
                    pl = pslp.tile([128, C], F32, tag="pl", name=f"pl{c}")
                    for i in range(4):
                        nc.tensor.matmul(
                            pl[:],
                            xT[i][:, 128 * j:128 * (j + 1)],
                            wst_t[i][:],
                            start=(i == 0),
                            stop=(i == 3),
                        )
                    ej = wp.tile([128, C], F16, tag="ej", name=f"ej{c}")
                    nc.scalar.activation(ej[:], pl[:], ACT.Exp)
                    tj = wp.tile([128, C], F16, tag="tj", name=f"tj{c}")
                    nc.vector.tensor_mul(tj[:], xsb_all[(sb, j // 8)][:, j % 8, :], ej[:])
                    chunk_et[c] = (ej, tj)
                # Col-tiled reduction batches.  Batch b reduces chunks
                # {b, b+4, b+8, b+12} of super-block dsb concurrently: chunk
                # 4m+b runs at PE column position 32m with stationary G32_b
                # (M=32, rows 8b..8b+8 of its window hold the chunk's 8
                # n-group sums, the rest are zeros that accumulate harmlessly).
                # Four fills overlap in the array, quartering reduction time.
                d = c - GDELAY
                if 0 <= d < TOTAL and d % NCHUNK >= 12:
                    dsb, b = divmod(d, NCHUNK)
                    b -= 12
                    if b == 0:
                        acc[dsb] = (
                            psa.tile([128, C], F32, tag="esum", name=f"esum{dsb}"),
                            psa.tile([128, C], F32, tag="tsum", name=f"tsum{dsb}"),
                        )
                    pe_sum, pt_sum = acc[dsb]
                    gb = g_t[:, 32 * b:32 * (b + 1)]
                    for which, dst in ((0, pe_sum), (1, pt_sum)):
                        if b == 0:
                            # Full-bank clear via a K=1 zero matmul: its writes
                            # land ~128 cycles before any concurrent col-tile's
                            # drain, so the start=True clear races nothing.
                            nc.tensor.matmul(
                                dst[:], zcol[:], zrow[:], start=True, stop=False
                            )
                        for m in range(4):
                            ch = NCHUNK * dsb + 4 * m + b
                            op = chunk_et[ch][which]
                            nc.tensor.matmul(
                                dst[32 * m:32 * (m + 1), :],
                                gb,
                                op[:],
                                start=False,
                                stop=(b == 3 and m == 3),
                                tile_position=(0, 32 * m),
                            )
                    for m in range(4):
                        del chunk_et[NCHUNK * dsb + 4 * m + b]
                    if b == 3:
                        steps = make_tail_steps(dsb)
                        for off, st in zip((1, 2, 3, 4, 5, 6, 7, 8, 10, 13), steps):
                            sched_steps.setdefault(c + off, []).append(st)
                for st in sched_steps.pop(c, []):
                    st()
    nc.compile()
    return nc


def _get_nc():
    if "nc" not in _cached:
        _cached["nc"] = _build()
    return _cached["nc"]


def _host_prep(x, w_score, w_conv, bn_gamma, bn_beta, bn_mean, bn_var):
    x = np.ascontiguousarray(np.asarray(x, dtype=np.float32)).reshape(B * N * K, C)
    w_score = np.asarray(w_score, dtype=np.float32)
    w_conv = np.asarray(w_conv, dtype=np.float32)
    inv = np.asarray(bn_gamma, dtype=np.float64) / np.sqrt(
        np.asarray(bn_var, dtype=np.float64) + BN_EPS
    )
    wc2 = w_conv.astype(np.float64) * inv[:, None]
    bias2 = (
        np.asarray(bn_beta, dtype=np.float64)
        - np.asarray(bn_mean, dtype=np.float64) * inv
    )
    # G32_b (cols 32b..32b+32): maps point pt to window-row 8b + pt//16.
    g = np.zeros((128, 128), dtype=np.float32)
    for b in range(4):
        for p in range(128):
            g[p, 32 * b + 8 * b + p // 16] = 1.0
    common = {
        "wst": np.ascontiguousarray(w_score.T).astype(ml_dtypes.bfloat16),
        "wc2t": np.ascontiguousarray(wc2.T).astype(np.float32),
        "bias2": bias2.reshape(1, COUT).astype(np.float32),
        "ones": np.ones((1, 128), dtype=np.float32),
        "gmat": g,
        "ident": np.eye(128, dtype=np.float32),
    }
    xb = x.astype(ml_dtypes.bfloat16)
    in_maps = []
    for c in range(NCORES):
        sl = slice(PTS_PER_CORE * c, PTS_PER_CORE * (c + 1))
        # host-transposed bf16: strip i holds channels 128i..128i+128 on the
        # partition axis so mm1's stationary loads need no on-device transpose.
        xbs = np.ascontiguousarray(xb[sl].T).reshape(4 * 128, PTS_PER_CORE)
        in_maps.append({"xf": x[sl], "xbT": xbs, **common})
    return in_maps


def kernel(x, w_score, w_conv, bn_gamma, bn_beta, bn_mean, bn_var):
    from concourse.bass_utils import run_bass_kernel_spmd

    nc = _get_nc()
    in_maps = _host_prep(x, w_score, w_conv, bn_gamma, bn_beta, bn_mean, bn_var)
    res = run_bass_kernel_spmd(nc, in_maps, core_ids=list(range(NCORES)))
    out = np.concatenate([res.results[c]["y"] for c in range(NCORES)], axis=0)
    return out.reshape(B, N, COUT).astype(np.float32)


# revision 25
# speedup vs baseline: 1.3210x; 1.0249x over previous
"""AttentionPooling TRN2 kernel: 8-core data-parallel over flattened (B*N) points.

Math (per point n with k=16 neighbors, C=512 channels):
  logits = x @ w_score.T            (per-channel attention logits)
  scores = softmax_k(logits)        (softmax over the k axis, per channel)
  pooled = sum_k x * scores
  y      = relu((pooled @ w_conv.T - mean) * gamma/sqrt(var+eps) + beta)

Device mapping (per core, 2048 n-points = 32768 (n,k) rows):
  - x rows (pt=(n,k) on partitions, c on free) feed the elementwise product.
  - mm1 uses bf16 copies of x transposed ON THE HOST (c on partitions) as the
    stationary operand: logits = xT.T @ w_score.T.  Device-side xbar
    transposes were the bottleneck (~64GB/s effective) and also corrupt
    under concurrent f32r/ACT-queue DMA traffic, so they are avoided.
  - softmax-over-k reductions run on the TensorEngine as matmuls with a
    0/1 group matrix G (k groups live in partition dim), accumulating 16
    chunks into one packed (128 n, 512 c) PSUM tile.
  - BN is folded into w_conv (scale) + a rank-1 bias matmul; ReLU on DVE.
  - fp16 is used for everything except the bf16 mm1 and fp32 accumulators:
    exact for the 0/1 G matrix, ~2^-11 rounding elsewhere, full-rate matmuls
    with overlappable weight loads, and 2x DVE modes.
"""
import numpy as np
import ml_dtypes

B, N, K, C, COUT = 4, 4096, 16, 512, 512
NCORES = 8
PTS_PER_CORE = B * N * K // NCORES      # 32768
NROWS_PER_CORE = B * N // NCORES        # 2048 n-points
NSB = NROWS_PER_CORE // 128             # 16 super-blocks of 128 n
NCHUNK = 16                             # chunks of 128 (n,k) rows per super-block
BN_EPS = 1e-5

_cached = {}


def _build():
    import concourse.bacc as bacc
    import concourse.mybir as mybir
    import concourse.tile as tile

    F32, F16, BF16 = mybir.dt.float32, mybir.dt.float16, mybir.dt.bfloat16
    ACT = mybir.ActivationFunctionType

    nc = bacc.Bacc("TRN2", target_bir_lowering=False, debug=False, num_devices=NCORES)
    xf = nc.dram_tensor("xf", [PTS_PER_CORE, C], F32, kind="ExternalInput")
    xbT = nc.dram_tensor("xbT", [4 * 128, PTS_PER_CORE], BF16, kind="ExternalInput")
    wst = nc.dram_tensor("wst", [C, C], BF16, kind="ExternalInput")
    wc2t = nc.dram_tensor("wc2t", [C, COUT], F32, kind="ExternalInput")
    bias2 = nc.dram_tensor("bias2", [1, COUT], F32, kind="ExternalInput")
    ones = nc.dram_tensor("ones", [1, 128], F32, kind="ExternalInput")
    gmat = nc.dram_tensor("gmat", [128, 128], F32, kind="ExternalInput")
    ident = nc.dram_tensor("ident", [128, 128], F32, kind="ExternalInput")
    y = nc.dram_tensor("y", [NROWS_PER_CORE, COUT], F32, kind="ExternalOutput")

    with tile.TileContext(nc) as tc:
        with (
            tc.tile_pool(name="const", bufs=1) as cp,
            tc.tile_pool(name="xT", bufs=3) as xtp,
            tc.tile_pool(name="xsb", bufs=2) as xsp,
            tc.tile_pool(name="work", bufs=20) as wp,
            tc.tile_pool(name="tail", bufs=2) as tp,
            tc.tile_pool(name="pl", bufs=3, space="PSUM") as pslp,
            tc.tile_pool(name="psacc", bufs=2, space="PSUM") as psa,
            tc.tile_pool(name="pstail", bufs=1, space="PSUM") as pst,
        ):
            # ---- constants (f16 ones produced via ACT copy from f32 staging) ----
            wst_t = [cp.tile([128, C], BF16, tag=f"wst{i}", name=f"wst{i}") for i in range(4)]
            for i in range(4):
                nc.sync.dma_start(wst_t[i][:], wst[128 * i:128 * (i + 1), :])
            wc2t_t = [cp.tile([128, COUT], F16, tag=f"wc2t{i}", name=f"wc2t{i}") for i in range(4)]
            bias2_t = cp.tile([1, COUT], F16, tag="bias2")
            ones_t = cp.tile([1, 128], F16, tag="ones")
            g_t = cp.tile([128, 128], F16, tag="g")
            id_t = cp.tile([128, 128], F16, tag="ident")
            zcol = cp.tile([1, 128], F16, tag="zcol")
            nc.vector.memset(zcol[:], 0.0)
            zrow = cp.tile([1, COUT], F16, tag="zrow")
            nc.vector.memset(zrow[:], 0.0)
            with tc.tile_pool(name="staging", bufs=1) as stp:
                wc2t_f = [stp.tile([128, COUT], F32, tag=f"wc2tf{i}", name=f"wc2tf{i}") for i in range(4)]
                for i in range(4):
                    nc.gpsimd.dma_start(wc2t_f[i][:], wc2t[128 * i:128 * (i + 1), :])
                    nc.scalar.copy(wc2t_t[i][:], wc2t_f[i][:])
                bias2_f = stp.tile([1, COUT], F32, tag="bias2f")
                nc.gpsimd.dma_start(bias2_f[:], bias2[:])
                nc.scalar.copy(bias2_t[:], bias2_f[:])
                ones_f = stp.tile([1, 128], F32, tag="onesf")
                nc.gpsimd.dma_start(ones_f[:], ones[:])
                nc.scalar.copy(ones_t[:], ones_f[:])
                g_f = stp.tile([128, 128], F32, tag="gf")
                nc.gpsimd.dma_start(g_f[:], gmat[:])
                nc.scalar.copy(g_t[:], g_f[:])
                id_f = stp.tile([128, 128], F32, tag="identf")
                nc.gpsimd.dma_start(id_f[:], ident[:])
                nc.scalar.copy(id_t[:], id_f[:])

            # Software-pipelined chunk loop over all NSB*NCHUNK chunks.
            # - G (reduction) matmuls for chunk c are emitted GDELAY chunks
            #   late so the PE never waits on that chunk's exp/mul chain.
            # - x (f32) is loaded in half-super-blocks (2 MB) with separate
            #   tile tags so slot recycling never delays the next prefetch.
            # - each super-block tail is emitted as small scheduled steps so
            #   the reciprocal never blocks the DVE FIFO for long and the PE
            #   tail matmuls land well after their inputs are ready.
            GDELAY = 6
            TOTAL = NSB * NCHUNK
            xT_all = {}
            xsb_all = {}
            chunk_et = {}
            acc = {}
            sched_steps = {}

            def issue_transposes(sb):
                xT = [
                    xtp.tile([128, 2048], BF16, tag=f"xT{i}", name=f"xT{i}_{sb}")
                    for i in range(4)
                ]
                row0 = 2048 * sb
                for i in range(4):
                    nc.sync.dma_start(
                        xT[i][:], xbT[128 * i:128 * (i + 1), row0:row0 + 2048]
                    )
                xT_all[sb] = xT

            def issue_xh(sb, h):
                row0 = 2048 * sb + 1024 * h
                xh = xsp.tile([128, 8, C], F32, tag=f"xsb{h}", name=f"xh{sb}_{h}")
                src_ap = xf[row0:row0 + 1024, :].rearrange("(ch p) c -> p ch c", p=128)
                nc.gpsimd.dma_start(xh[:], src_ap)
                xsb_all[(sb, h)] = xh

            def make_tail_steps(sb):
                pe_sum, pt_sum = acc.pop(sb)
                inv_t = tp.tile([128, C], F32, tag="inv", name=f"inv{sb}")
                pooled = tp.tile([128, C], F16, tag="pooled", name=f"pooled{sb}")
                state = {}

                def norm_step(i):
                    def f():
                        sl = slice(64 * i, 64 * (i + 1))
                        nc.vector.reciprocal(inv_t[:, sl], pe_sum[:, sl])
                        nc.vector.tensor_mul(pooled[:, sl], pt_sum[:, sl], inv_t[:, sl])
                    return f

                def transpose_step():
                    ppT = pst.tile([128, C], F16, tag="pstail", name=f"ppT{sb}")
                    for i in range(4):
                        nc.tensor.transpose(
                            ppT[:, 128 * i:128 * (i + 1)],
                            pooled[:, 128 * i:128 * (i + 1)],
                            id_t[:],
                        )
                    pT = tp.tile([128, C], F16, tag="pT", name=f"pT{sb}")
                    nc.vector.tensor_copy(pT[:], ppT[:])
                    state["pT"] = pT

                def mm2_step():
                    pT = state["pT"]
                    py = pst.tile([128, COUT], F32, tag="pstail", name=f"py{sb}")
                    for i in range(4):
                        nc.tensor.matmul(
                            py[:],
                            pT[:, 128 * i:128 * (i + 1)],
                            wc2t_t[i][:],
                            start=(i == 0),
                            stop=False,
                        )
                    nc.tensor.matmul(
                        py[:], ones_t[:], bias2_t[:], start=False, stop=True
                    )
                    y_t = tp.tile([128, COUT], F32, tag="yt", name=f"yt{sb}")
                    nc.vector.tensor_scalar_max(y_t[:], py[:], 0.0)
                    nc.gpsimd.dma_start(y[128 * sb:128 * (sb + 1), :], y_t[:])

                return [norm_step(i) for i in range(8)] + [transpose_step, mm2_step]



            issue_transposes(0)
            issue_transposes(1)
            issue_transposes(2)
            issue_xh(0, 0)
            issue_xh(0, 1)
            for c in range(TOTAL + GDELAY + 16):
                sb, j = divmod(c, NCHUNK)
                if c < TOTAL:
                    if j == 0 and sb + 1 < NSB:
                        issue_xh(sb + 1, 0)
                    if j == 8 and sb + 1 < NSB:
                        issue_xh(sb + 1, 1)
                    if j == 1 and sb + 3 < NSB:
                        issue_transposes(sb + 3)
                    xT = xT_all[sb]
                    pl = pslp.tile([128, C], F32, tag="pl", name=f"pl{c}")
                    for i in range(4):
                        nc.tensor.matmul(
                            pl[:],
                            xT[i][:, 128 * j:128 * (j + 1)],
                            wst_t[i][:],
                            start=(i == 0),
                            stop=(i == 3),
                        )
                    ej = wp.tile([128, C], F16, tag="ej", name=f"ej{c}")
                    nc.scalar.activation(ej[:], pl[:], ACT.Exp)
                    tj = wp.tile([128, C], F16, tag="tj", name=f"tj{c}")
                    nc.vector.tensor_mul(tj[:], xsb_all[(sb, j // 8)][:, j % 8, :], ej[:])
                    chunk_et[c] = (ej, tj)
                # Col-tiled reduction batches.  Batch b reduces chunks
                # {b, b+4, b+8, b+12} of super-block dsb concurrently: chunk
                # 4m+b runs at PE column position 32m with stationary G32_b
                # (M=32, rows 8b..8b+8 of its window hold the chunk's 8
                # n-group sums, the rest are zeros that accumulate harmlessly).
                # Four fills overlap in the array, quartering reduction time.
                d = c - GDELAY
                if 0 <= d < TOTAL and d % NCHUNK >= 12:
                    dsb, b = divmod(d, NCHUNK)
                    b -= 12
                    if b == 0:
                        acc[dsb] = (
                            psa.tile([128, C], F32, tag="esum", name=f"esum{dsb}"),
                            psa.tile([128, C], F32, tag="tsum", name=f"tsum{dsb}"),
                        )
                    pe_sum, pt_sum = acc[dsb]
                    gb = g_t[:, 32 * b:32 * (b + 1)]
                    for which, dst in ((0, pe_sum), (1, pt_sum)):
                        if b == 0:
                            # Full-bank clear via a K=1 zero matmul: its writes
                            # land ~128 cycles before any concurrent col-tile's
                            # drain, so the start=True clear races nothing.
                            nc.tensor.matmul(
                                dst[:], zcol[:], zrow[:], start=True, stop=False
                            )
                        for m in range(4):
                            ch = NCHUNK * dsb + 4 * m + b
                            op = chunk_et[ch][which]
                            nc.tensor.matmul(
                                dst[32 * m:32 * (m + 1), :],
                                gb,
                                op[:],
                                start=False,
                                stop=(b == 3 and m == 3),
                                tile_position=(0, 32 * m),
                            )
                    for m in range(4):
                        del chunk_et[NCHUNK * dsb + 4 * m + b]
                    if b == 3:
                        steps = make_tail_steps(dsb)
                        for off, st in zip((1, 2, 3, 4, 5, 6, 7, 8, 10, 13), steps):
                            sched_steps.setdefault(c + off, []).append(st)
                for st in sched_steps.pop(c, []):
                    st()
    nc.compile()
    return nc


def _get_nc():
    if "nc" not in _cached:
        _cached["nc"] = _build()
    return _cached["nc"]


def _host_prep(x, w_score, w_conv, bn_gamma, bn_beta, bn_mean, bn_var):
    x = np.ascontiguousarray(np.asarray(x, dtype=np.float32)).reshape(B * N * K, C)
    w_score = np.asarray(w_score, dtype=np.float32)
    w_conv = np.asarray(w_conv, dtype=np.float32)
    inv = np.asarray(bn_gamma, dtype=np.float64) / np.sqrt(
        np.asarray(bn_var, dtype=np.float64) + BN_EPS
    )
    wc2 = w_conv.astype(np.float64) * inv[:, None]
    bias2 = (
        np.asarray(bn_beta, dtype=np.float64)
        - np.asarray(bn_mean, dtype=np.float64) * inv
    )
    # G32_b (cols 32b..32b+32): maps point pt to window-row 8b + pt//16.
    g = np.zeros((128, 128), dtype=np.float32)
    for b in range(4):
        for p in range(128):
            g[p, 32 * b + 8 * b + p // 16] = 1.0
    common = {
        "wst": np.ascontiguousarray(w_score.T).astype(ml_dtypes.bfloat16),
        "wc2t": np.ascontiguousarray(wc2.T).astype(np.float32),
        "bias2": bias2.reshape(1, COUT).astype(np.float32),
        "ones": np.ones((1, 128), dtype=np.float32),
        "gmat": g,
        "ident": np.eye(128, dtype=np.float32),
    }
    xb = x.astype(ml_dtypes.bfloat16)
    in_maps = []
    for c in range(NCORES):
        sl = slice(PTS_PER_CORE * c, PTS_PER_CORE * (c + 1))
        # host-transposed bf16: strip i holds channels 128i..128i+128 on the
        # partition axis so mm1's stationary loads need no on-device transpose.
        xbs = np.ascontiguousarray(xb[sl].T).reshape(4 * 128, PTS_PER_CORE)
        in_maps.append({"xf": x[sl], "xbT": xbs, **common})
    return in_maps


def kernel(x, w_score, w_conv, bn_gamma, bn_beta, bn_mean, bn_var):
    from concourse.bass_utils import run_bass_kernel_spmd

    nc = _get_nc()
    in_maps = _host_prep(x, w_score, w_conv, bn_gamma, bn_beta, bn_mean, bn_var)
    res = run_bass_kernel_spmd(nc, in_maps, core_ids=list(range(NCORES)))
    out = np.concatenate([res.results[c]["y"] for c in range(NCORES)], axis=0)
    return out.reshape(B, N, COUT).astype(np.float32)
